# revision 9
# baseline (speedup 1.0000x reference)
"""Trainium2 Bass kernel for BinaryMemoryRNN (scatter_memory).

Math (per batch row b):
    logits = h_prev @ M_w.T + M_b                 [B, 10]
    bits   = (sigmoid(logits) > 0.5) = (logits > -M_b)
    index  = sum(bits * 2^(9-i))                  [B] in [0, 1023]
    h_mem  = memory[index]
    pre    = x @ W_w.T + W_b + h_prev @ U_w.T + U_b + h_mem @ Q_w.T + Q_b
    out    = sigmoid(LayerNorm(pre) * ln_g + ln_b)

Key transforms:
  * h_mem @ Q_w.T == (memory @ Q_w.T)[index], so R = memory @ Q_w.T + bias
    is precomputed once per core ([1024, 1024]) and the gather+matmul becomes
    a row gather of R feeding an add.
  * The three big matmuls (x@W.T, h@U.T, memory@Q.T) run in fp8 e4m3 with
    DoubleRow perf mode (2x PE throughput vs bf16). Weights are scaled by 16
    on the host so their ~0.02-magnitude entries stay in e4m3's normal range;
    the 1/16 descale is fused into the PSUM-evacuating add. Simulated end to
    end this lands at rel err ~1.2e-2 vs the fp32 reference (gate: 2e-2).
  * The address logits must be near-fp32 exact (a flipped bit selects a
    completely different memory row, ~2.5e-3 rel err per flip). They are
    computed as (h_hi + h_lo) @ (M_hi + M_lo) with all four factors bf16 and
    fp32 PSUM accumulation: bf16 products are exact in fp32, so the only loss
    is the h_lo rounding (~2^-17 relative) - simulation shows 0 flipped
    indices. Grouped transposed form: lhsT = [M_hi | M_lo] stacked [128, 20],
    rhs = 512 batch columns, accumulating h_hi then h_lo streams into a
    [20, 512] PSUM tile; logits = rows 0:10 + rows 10:20.
  * Indices come out of a [128, 512] bits tile (rows 0:10 live, rows 10:128
    zeroed once) via a standard K=128 matmul against a zero-padded powers
    vector - one [128, 1] per-partition index tile per batch tile, no
    transposes, no small-K matmuls.

Sharding: data-parallel over batch across 8 cores (2048 rows each);
weights + memory table replicated. All operands are pre-transposed /
pre-tiled on the host so the device does zero transposes:
  - fp8 activations as [kp, bt, c4, ki2, bp] tiles (K = c*256 + ki*128 + kp)
  - fp8 weights as [kp, c4, ki2, n]
  - bf16 logit activations as [kp, g, kc, tb, bp] (one 8KB/partition DMA
    per 4-tile group), logit weights as [kp, kc8, 20]
Output is written bf16 (halves output DMA) and cast to fp32 on the host.
"""

import numpy as np
import ml_dtypes
from contextlib import ExitStack

import concourse.bass as bass
import concourse.mybir as mybir
import concourse.tile as tile
from concourse import bacc
from concourse import bass_utils

P = 128            # partitions
NCORES = 8
B = 16384          # full batch
BC = B // NCORES   # batch rows per core (2048)
BT = BC // P       # b-tiles per core (16)
C4 = 4             # fp8 DoubleRow contraction chunks (1024 / 256)
KC = 8             # bf16 contraction chunks (1024 / 128)
H = 1024
NB = 10            # address bits
MEM = 1024         # memory rows
LN_EPS = 1e-5
GROUP = 4          # b-tiles per logits group
NG = BT // GROUP   # logits groups (4)
WS = 16.0          # host-side weight scale for fp8

F32 = mybir.dt.float32
BF16 = mybir.dt.bfloat16
F8 = mybir.dt.float8e4
I32 = mybir.dt.int32
BF16_NP = ml_dtypes.bfloat16
F8_NP = ml_dtypes.float8_e4m3
DR = mybir.MatmulPerfMode.DoubleRow

_CACHE = {}


def _bcast_ap(handle, n):
    """[n] DRAM tensor -> [P, n] AP broadcast across partitions (step 0)."""
    h = handle.ap()
    return bass.AP(tensor=h.tensor, offset=h.offset, ap=[[0, P], *list(h.ap)])


def build_nc(zero_affine=True, warmup=True):
    nc = bacc.Bacc("TRN2", debug=False, enable_asserts=False)

    x8 = nc.dram_tensor("x8", [P, BT, C4, 2, P], F8, kind="ExternalInput")
    h8 = nc.dram_tensor("h8", [P, BT, C4, 2, P], F8, kind="ExternalInput")
    hh = nc.dram_tensor("hh", [P, NG, KC, GROUP, P], BF16, kind="ExternalInput")
    hl = nc.dram_tensor("hl", [P, NG, KC, GROUP, P], BF16, kind="ExternalInput")
    W8 = nc.dram_tensor("W8", [P, C4, 2, H], F8, kind="ExternalInput")
    U8 = nc.dram_tensor("U8", [P, C4, 2, H], F8, kind="ExternalInput")
    Q8 = nc.dram_tensor("Q8", [P, C4, 2, H], F8, kind="ExternalInput")
    M8 = nc.dram_tensor("M8", [P, C4, 2, MEM], F8, kind="ExternalInput")
    # [M_hi | zeros | M_lo] stacked columns: M_lo lands at partition 32 of
    # the logits PSUM tile (engine reads must start at partition 0/32/64/96).
    MSTK = 32 + NB
    Mstk = nc.dram_tensor("Mstk", [P, KC, MSTK], BF16, kind="ExternalInput")
    cb = nc.dram_tensor("cb", [H], F32, kind="ExternalInput")
    lng = nc.dram_tensor("lng", [H], F32, kind="ExternalInput")
    lnb = nc.dram_tensor("lnb", [H], F32, kind="ExternalInput")
    negmb = nc.dram_tensor("negmb", [NB], F32, kind="ExternalInput")
    powers = nc.dram_tensor("powers", [NB], F32, kind="ExternalInput")
    y = nc.dram_tensor("y", [BC, H], BF16, kind="ExternalOutput")
    R = nc.dram_tensor("Rtab", [MEM, H], BF16, kind="Internal")
    wsink_d = nc.dram_tensor("wsink", [P, 1], F32, kind="Internal")
    y_ap = y.ap()
    R_ap = R.ap()

    INV = 1.0 / WS

    with tile.TileContext(nc) as tc, ExitStack() as ctx:
        wpool = ctx.enter_context(tc.tile_pool(name="weights", bufs=1))
        work = ctx.enter_context(tc.tile_pool(name="work", bufs=4))
        hpool = ctx.enter_context(tc.tile_pool(name="hpool", bufs=2))
        epil = ctx.enter_context(tc.tile_pool(name="epil", bufs=GROUP))
        small = ctx.enter_context(tc.tile_pool(name="small", bufs=2 * GROUP + 2))
        psum = ctx.enter_context(tc.tile_pool(name="psum", bufs=2, space="PSUM"))

        wu_l = wpool.tile([P, P], BF16)
        wu_r = wpool.tile([P, 512], BF16)
        nc.vector.memset(wu_l[:], 0)
        nc.vector.memset(wu_r[:], 0)

        def pe_heat(n, tag):
            """n back-to-back dummy matmuls (one accumulation group) to keep
            the HAM clock gate at K=8/8 while the PE has no real work."""
            ps_w = psum.tile([P, 512], F32, tag="psT", space="PSUM", bufs=1,
                             name=tag)
            for i in range(n):
                nc.tensor.matmul(out=ps_w[:], lhsT=wu_l[:], rhs=wu_r[:],
                                 start=(i == 0), stop=(i == n - 1))
            wsink = wpool.tile([P, 1], F32, name=f"ws_{tag}")
            nc.vector.tensor_copy(out=wsink[:], in_=ps_w[:, 0:1])
            nc.sync.dma_start(out=wsink_d.ap()[:, :], in_=wsink[:])

        if warmup:
            # garbage SBUF values are fine; the sink result is discarded
            pe_heat(20, "wu0")

        # ---- resident constants; R operands chunked so matmuls start early
        m8_sb = wpool.tile([P, C4, 2, MEM], F8)
        q8_sb = wpool.tile([P, C4, 2, H], F8)
        w8_sb = wpool.tile([P, C4, 2, H], F8)
        u8_sb = wpool.tile([P, C4, 2, H], F8)
        mstk_sb = wpool.tile([P, KC, MSTK], BF16)
        nc.sync.dma_start(out=mstk_sb[:], in_=Mstk.ap()[:, :, :])
        for c in range(C4):
            nc.sync.dma_start(out=m8_sb[:, c, :, :], in_=M8.ap()[:, c, :, :])
            nc.sync.dma_start(out=q8_sb[:, c, :, :], in_=Q8.ap()[:, c, :, :])

        def prefetch_group(g):
            hgh = hpool.tile([P, KC, GROUP, P], BF16, tag="hgh")
            hgl = hpool.tile([P, KC, GROUP, P], BF16, tag="hgl")
            nc.sync.dma_start(out=hgh[:], in_=hh.ap()[:, g, :, :, :])
            nc.sync.dma_start(out=hgl[:], in_=hl.ap()[:, g, :, :, :])
            return hgh, hgl

        # group-0 logits activations land before the W/U weights: the PE is
        # busy with R until then, and tile 0 consumes W/U only after logits 0
        g_tiles = prefetch_group(0)
        for c in range(C4):
            nc.sync.dma_start(out=w8_sb[:, c, :, :], in_=W8.ap()[:, c, :, :])
        for c in range(C4):
            nc.sync.dma_start(out=u8_sb[:, c, :, :], in_=U8.ap()[:, c, :, :])

        cbb = wpool.tile([P, H], F32)
        nc.gpsimd.dma_start(out=cbb[:], in_=_bcast_ap(cb, H))
        nmb_c = wpool.tile([NB, 1], F32)
        nc.sync.dma_start(out=nmb_c[:], in_=negmb.ap()[:, None])
        pw128 = wpool.tile([P, 1], F32)
        nc.vector.memset(pw128[:], 0)
        nc.sync.dma_start(out=pw128[0:NB, :], in_=powers.ap()[:, None])
        bits = wpool.tile([P, 512], F32)
        nc.vector.memset(bits[:], 0)
        if not zero_affine:
            gb = wpool.tile([P, H], F32)
            bb = wpool.tile([P, H], F32)
            nc.gpsimd.dma_start(out=gb[:], in_=_bcast_ap(lng, H))
            nc.gpsimd.dma_start(out=bb[:], in_=_bcast_ap(lnb, H))
            eps = wpool.tile([P, 1], F32)
            nc.vector.memset(eps[:], LN_EPS)

        # ---- phase 1: R = (memory @ Q_w.T)/WS + combined_bias -> DRAM bf16
        for mt in range(KC):
            psA = psum.tile([P, 512], F32, tag="psA", space="PSUM", bufs=3)
            psB = psum.tile([P, 512], F32, tag="psB", space="PSUM", bufs=3)
            for c in range(C4):
                lhs = m8_sb[:, c, :, mt * P:(mt + 1) * P]
                nc.tensor.matmul(out=psA[:], lhsT=lhs,
                                 rhs=q8_sb[:, c, :, 0:512],
                                 start=(c == 0), stop=(c == C4 - 1),
                                 perf_mode=DR)
                nc.tensor.matmul(out=psB[:], lhsT=lhs,
                                 rhs=q8_sb[:, c, :, 512:1024],
                                 start=(c == 0), stop=(c == C4 - 1),
                                 perf_mode=DR)
            r_sb = work.tile([P, H], BF16, tag="rtile")
            nc.vector.scalar_tensor_tensor(out=r_sb[:, 0:512], in0=psA[:],
                                           scalar=INV, in1=cbb[:, 0:512],
                                           op0=mybir.AluOpType.mult,
                                           op1=mybir.AluOpType.add)
            nc.vector.scalar_tensor_tensor(out=r_sb[:, 512:1024], in0=psB[:],
                                           scalar=INV, in1=cbb[:, 512:1024],
                                           op0=mybir.AluOpType.mult,
                                           op1=mybir.AluOpType.add)
            nc.gpsimd.dma_start(out=R_ap[mt * P:(mt + 1) * P, :], in_=r_sb[:])

        # ---- phase 2 ----
        def logits_group(g, tiles):
            """Exact fp32 logits for GROUP b-tiles -> [P, GROUP] index tile."""
            hgh, hgl = tiles
            psLT = psum.tile([MSTK, 512], F32, tag="psL", space="PSUM",
                             bufs=1)
            for kc in range(KC):
                nc.tensor.matmul(out=psLT[:], lhsT=mstk_sb[:, kc, :],
                                 rhs=hgh[:, kc, :, :], start=(kc == 0),
                                 stop=False)
            for kc in range(KC):
                nc.tensor.matmul(out=psLT[:], lhsT=mstk_sb[:, kc, :],
                                 rhs=hgl[:, kc, :, :], start=False,
                                 stop=(kc == KC - 1))
            # DVE may read only one PSUM operand per op: stage the M_lo rows
            # through SBUF, then add against the M_hi rows still in PSUM.
            lo_sb = small.tile([NB, 512], F32, tag="lo_sb")
            nc.vector.tensor_copy(out=lo_sb[:], in_=psLT[32:32 + NB, :])
            lg = small.tile([NB, 512], F32, tag="lg")
            nc.vector.tensor_tensor(out=lg[:], in0=psLT[0:NB, :],
                                    in1=lo_sb[:],
                                    op=mybir.AluOpType.add)
            # bits rows 0:10 (rows 10:128 pre-zeroed; pw128 rows 10:128 = 0)
            nc.vector.tensor_scalar(out=bits[0:NB, :], in0=lg[:],
                                    scalar1=nmb_c[:], scalar2=None,
                                    op0=mybir.AluOpType.is_gt)
            # per-tile indices into 4 columns of one PSUM tile (no PE<->DVE
            # ping-pong), one i32 copy for the whole group
            pst = psum.tile([P, GROUP], F32, tag="psT", space="PSUM", bufs=1)
            for tb in range(GROUP):
                nc.tensor.matmul(out=pst[:, tb:tb + 1],
                                 lhsT=bits[:, tb * P:(tb + 1) * P],
                                 rhs=pw128[:], start=True, stop=True)
            idx = small.tile([P, GROUP], I32, tag="idx")
            nc.vector.tensor_copy(out=idx[:], in_=pst[:])
            return idx

        def stage_a(bt, idx_ap):
            xb = work.tile([P, C4, 2, P], F8, tag="xb")
            hb = work.tile([P, C4, 2, P], F8, tag="hb")
            nc.sync.dma_start(out=xb[:], in_=x8.ap()[:, bt, :, :, :])
            nc.sync.dma_start(out=hb[:], in_=h8.ap()[:, bt, :, :, :])

            rg = work.tile([P, H], BF16, tag="rg")
            nc.gpsimd.indirect_dma_start(
                out=rg[:], out_offset=None, in_=R_ap[:, :],
                in_offset=bass.IndirectOffsetOnAxis(ap=idx_ap, axis=0))

            ps0 = psum.tile([P, 512], F32, tag="psA", space="PSUM", bufs=3)
            ps1 = psum.tile([P, 512], F32, tag="psB", space="PSUM", bufs=3)
            for c in range(C4):
                nc.tensor.matmul(out=ps0[:], lhsT=xb[:, c, :, :],
                                 rhs=w8_sb[:, c, :, 0:512],
                                 start=(c == 0), stop=False, perf_mode=DR)
                nc.tensor.matmul(out=ps1[:], lhsT=xb[:, c, :, :],
                                 rhs=w8_sb[:, c, :, 512:1024],
                                 start=(c == 0), stop=False, perf_mode=DR)
            for c in range(C4):
                nc.tensor.matmul(out=ps0[:], lhsT=hb[:, c, :, :],
                                 rhs=u8_sb[:, c, :, 0:512],
                                 start=False, stop=(c == C4 - 1), perf_mode=DR)
                nc.tensor.matmul(out=ps1[:], lhsT=hb[:, c, :, :],
                                 rhs=u8_sb[:, c, :, 512:1024],
                                 start=False, stop=(c == C4 - 1), perf_mode=DR)

            pre = epil.tile([P, H], F32, tag="pre")
            nc.vector.scalar_tensor_tensor(out=pre[:, 0:512], in0=ps0[:],
                                           scalar=INV, in1=rg[:, 0:512],
                                           op0=mybir.AluOpType.mult,
                                           op1=mybir.AluOpType.add)
            nc.vector.scalar_tensor_tensor(out=pre[:, 512:1024], in0=ps1[:],
                                           scalar=INV, in1=rg[:, 512:1024],
                                           op0=mybir.AluOpType.mult,
                                           op1=mybir.AluOpType.add)

            stats = small.tile([P, 2, 6], F32, tag="stats")
            mv = small.tile([P, 2], F32, tag="mv")
            nc.vector.bn_stats(out=stats[:, 0, :], in_=pre[:, 0:512])
            nc.vector.bn_stats(out=stats[:, 1, :], in_=pre[:, 512:1024])
            nc.vector.bn_aggr(out=mv[:], in_=stats[:])

            if zero_affine:
                # rstd via bit-trick + 2 Newton steps (keeps Scalar engine's
                # activation table pinned to Sigmoid); then one fused
                # activation: sigmoid(pre * rstd - mu * rstd).
                v = small.tile([P, 1], F32, tag="v")
                ri = small.tile([P, 1], I32, tag="ri")
                t = small.tile([P, 1], F32, tag="t")
                nmr = small.tile([P, 1], F32, tag="nmr")
                ry = ri[:].bitcast(F32)
                nc.vector.tensor_scalar_add(out=v[:], in0=mv[:, 1:2],
                                            scalar1=LN_EPS)
                nc.vector.tensor_scalar(out=ri[:], in0=v[:].bitcast(I32),
                                        scalar1=1, scalar2=None,
                                        op0=mybir.AluOpType.arith_shift_right)
                nc.vector.tensor_scalar(out=ri[:], in0=ri[:], scalar1=0,
                                        scalar2=None,
                                        op0=mybir.AluOpType.bitwise_not)
                nc.vector.tensor_scalar(out=ri[:], in0=ri[:],
                                        scalar1=0x5F3759E0, scalar2=None,
                                        op0=mybir.AluOpType.add)
                for _ in range(2):
                    nc.vector.tensor_tensor(out=t[:], in0=ry, in1=ry,
                                            op=mybir.AluOpType.mult)
                    nc.vector.tensor_tensor(out=t[:], in0=t[:], in1=v[:],
                                            op=mybir.AluOpType.mult)
                    nc.vector.tensor_scalar(out=t[:], in0=t[:], scalar1=-0.5,
                                            scalar2=1.5,
                                            op0=mybir.AluOpType.mult,
                                            op1=mybir.AluOpType.add)
                    nc.vector.tensor_tensor(out=ry, in0=ry, in1=t[:],
                                            op=mybir.AluOpType.mult)
                nc.vector.scalar_tensor_tensor(out=nmr[:], in0=mv[:, 0:1],
                                               scalar=-1.0, in1=ry,
                                               op0=mybir.AluOpType.mult,
                                               op1=mybir.AluOpType.mult)
                ob = work.tile([P, H], BF16, tag="ob")
                nc.scalar.activation(out=ob[:], in_=pre[:],
                                     func=mybir.ActivationFunctionType.Sigmoid,
                                     bias=nmr[:], scale=ri[:].bitcast(F32))
                nc.scalar.dma_start(out=y_ap[bt * P:(bt + 1) * P, :],
                                    in_=ob[:])
                return None

            sd = small.tile([P, 1], F32, tag="sd")
            rstd = small.tile([P, 1], F32, tag="rstd")
            nc.scalar.activation(out=sd[:], in_=mv[:, 1:2],
                                 func=mybir.ActivationFunctionType.Sqrt,
                                 bias=eps[:], scale=1.0)
            nc.vector.reciprocal(out=rstd[:], in_=sd[:])
            return pre, mv, rstd

        def stage_b(bt, pre, mv, rstd):
            nc.vector.scalar_tensor_tensor(out=pre[:], in0=pre[:],
                                           scalar=mv[:, 0:1], in1=gb[:],
                                           op0=mybir.AluOpType.subtract,
                                           op1=mybir.AluOpType.mult)
            nc.vector.scalar_tensor_tensor(out=pre[:], in0=pre[:],
                                           scalar=rstd[:], in1=bb[:],
                                           op0=mybir.AluOpType.mult,
                                           op1=mybir.AluOpType.add)
            ob = work.tile([P, H], BF16, tag="ob")
            nc.scalar.activation(out=ob[:], in_=pre[:],
                                 func=mybir.ActivationFunctionType.Sigmoid)
            nc.scalar.dma_start(out=y_ap[bt * P:(bt + 1) * P, :], in_=ob[:])

        for g in range(NG):
            idx = logits_group(g, g_tiles)
            staged = []
            for tb in range(GROUP):
                r = stage_a(g * GROUP + tb, idx[:, tb:tb + 1])
                if r is not None:
                    staged.append((g * GROUP + tb, *r))
                if tb == 0 and g + 1 < NG:
                    g_tiles = prefetch_group(g + 1)
            for bt, pre, mv, rstd in staged:
                stage_b(bt, pre, mv, rstd)
        if warmup:
            # hold the clock at K=8/8 while the last epilogues drain
            pe_heat(24, "wu1")

    nc.compile()
    return nc


def _get_nc(zero_affine=True):
    key = ("nc", zero_affine)
    if key not in _CACHE:
        _CACHE[key] = build_nc(zero_affine)
    return _CACHE[key]


def _tile_act8(a):
    """[BC, 1024] f32 -> [kp, bt, c4, ki2, bp] fp8, k = c*256 + ki*128 + kp."""
    t = a.reshape(BT, P, C4, 2, P).transpose(4, 0, 2, 3, 1)
    return np.ascontiguousarray(t).astype(F8_NP)


def _tile_w8(w):
    """[n, 1024] f32 (contraction on axis 1) -> [kp, c4, ki2, n] fp8."""
    t = w.T.reshape(C4, 2, P, -1).transpose(2, 0, 1, 3)
    return np.ascontiguousarray(t).astype(F8_NP)


def _tile_hpair(a):
    """[BC, 1024] bf16 -> [kp, g, kc, tb, bp], k = kc*128 + kp."""
    t = a.reshape(NG, GROUP, P, KC, P).transpose(4, 0, 3, 1, 2)
    return np.ascontiguousarray(t)


def prepare_in_maps(inputs):
    x = np.asarray(inputs["x"], np.float32)
    h = np.asarray(inputs["h_prev"], np.float32)
    memory = np.asarray(inputs["memory"], np.float32)
    W_w = np.asarray(inputs["W_w"], np.float32)
    U_w = np.asarray(inputs["U_w"], np.float32)
    Q_w = np.asarray(inputs["Q_w"], np.float32)
    M_w = np.asarray(inputs["M_w"], np.float32)
    W_b = np.asarray(inputs["W_b"], np.float32)
    U_b = np.asarray(inputs["U_b"], np.float32)
    Q_b = np.asarray(inputs["Q_b"], np.float32)
    M_b = np.asarray(inputs["M_b"], np.float32)
    ln_g = np.asarray(inputs["ln_g"], np.float32)
    ln_b = np.asarray(inputs["ln_b"], np.float32)

    M_hi = M_w.astype(BF16_NP).astype(np.float32)
    M_lo = (M_w - M_hi).astype(BF16_NP).astype(np.float32)
    # [kp, kc, 42] = [M_hi | zeros(22) | M_lo] columns, k = kc*128 + kp
    mstk = np.concatenate(
        [M_hi.T, np.zeros((H, 22), np.float32), M_lo.T], axis=1)  # [1024, 42]
    mstk = mstk.reshape(KC, P, 42).transpose(1, 0, 2)

    shared = {
        "W8": _tile_w8(W_w * WS),
        "U8": _tile_w8(U_w * WS),
        "Q8": _tile_w8(Q_w * WS),
        # contraction for R = memory @ Q_w.T is over memory's axis 1 (HIDDEN);
        # rows (axis 0) are the "out" dim -> same transform as W.
        "M8": _tile_w8(memory),
        "Mstk": np.ascontiguousarray(mstk).astype(BF16_NP),
        "cb": np.ascontiguousarray(W_b + U_b + Q_b),
        "lng": np.ascontiguousarray(ln_g),
        "lnb": np.ascontiguousarray(ln_b),
        "negmb": np.ascontiguousarray(-M_b),
        "powers": (2.0 ** np.arange(NB - 1, -1, -1)).astype(np.float32),
    }
    in_maps = []
    for i in range(NCORES):
        sl = slice(i * BC, (i + 1) * BC)
        hs = h[sl]
        h_hi = hs.astype(BF16_NP)
        h_lo = (hs - h_hi.astype(np.float32)).astype(BF16_NP)
        m = dict(shared)
        m["x8"] = _tile_act8(x[sl])
        m["h8"] = _tile_act8(hs)
        m["hh"] = _tile_hpair(h_hi)
        m["hl"] = _tile_hpair(h_lo)
        in_maps.append(m)
    return in_maps


def run(inputs, trace=False, trace_cores=None):
    zero_affine = bool(
        np.all(np.asarray(inputs["ln_g"], np.float32) == 1.0)
        and np.all(np.asarray(inputs["ln_b"], np.float32) == 0.0))
    nc = _get_nc(zero_affine)
    in_maps = prepare_in_maps(inputs)
    res = bass_utils.run_bass_kernel_spmd(
        nc, in_maps, core_ids=list(range(NCORES)), trace=trace,
        trace_cores=trace_cores)
    out = np.concatenate(
        [np.asarray(r["y"]).astype(np.float32) for r in res.results], axis=0)
    return out, res


def kernel(**inputs):
    out, _ = run(inputs)
    return out.astype(np.float32)


def enable_profiling():
    """Inject the missing antenv.axon_hooks shim so trace=True works, and
    neutralize the S3 artifact upload (zero-egress container)."""
    import sys
    import types
    try:
        import antenv.axon_hooks  # noqa: F401
    except ImportError:
        mod = types.ModuleType("antenv.axon_hooks")
        _hook = [None]
        mod.set_axon_ntff_profile_hook = lambda h: _hook.__setitem__(0, h)
        mod.get_axon_ntff_profile_hook = lambda: _hook[0]
        sys.modules["antenv.axon_hooks"] = mod
        from trn_agent_boot.trn_boot import _ntff_profile_via_ctypes
        mod.set_axon_ntff_profile_hook(
            _ntff_profile_via_ctypes("/opt/axon/libaxon_pjrt.so"))
    bass_utils.upload_artifacts = lambda d: "local://" + str(d)


# revision 10
# speedup vs baseline: 1.1356x; 1.1356x over previous
"""Trainium2 Bass kernel for BinaryMemoryRNN (scatter_memory).

Math (per batch row b):
    logits = h_prev @ M_w.T + M_b                 [B, 10]
    bits   = (sigmoid(logits) > 0.5) = (logits > -M_b)
    index  = sum(bits * 2^(9-i))                  [B] in [0, 1023]
    h_mem  = memory[index]
    pre    = x @ W_w.T + W_b + h_prev @ U_w.T + U_b + h_mem @ Q_w.T + Q_b
    out    = sigmoid(LayerNorm(pre) * ln_g + ln_b)

Key transforms:
  * h_mem @ Q_w.T == (memory @ Q_w.T)[index], so R = memory @ Q_w.T + bias
    is precomputed once per core ([1024, 1024]) and the gather+matmul becomes
    a row gather of R feeding an add.
  * The three big matmuls (x@W.T, h@U.T, memory@Q.T) run in fp8 e4m3 with
    DoubleRow perf mode (2x PE throughput vs bf16). Weights are scaled by 16
    on the host so their ~0.02-magnitude entries stay in e4m3's normal range;
    the 1/16 descale is fused into the PSUM-evacuating add. Simulated end to
    end this lands at rel err ~1.2e-2 vs the fp32 reference (gate: 2e-2).
  * The address logits must be near-fp32 exact (a flipped bit selects a
    completely different memory row, ~2.5e-3 rel err per flip). They are
    computed as (h_hi + h_lo) @ (M_hi + M_lo) with all four factors bf16 and
    fp32 PSUM accumulation: bf16 products are exact in fp32, so the only loss
    is the h_lo rounding (~2^-17 relative) - simulation shows 0 flipped
    indices. Grouped transposed form: lhsT = [M_hi | M_lo] stacked [128, 20],
    rhs = 512 batch columns, accumulating h_hi then h_lo streams into a
    [20, 512] PSUM tile; logits = rows 0:10 + rows 10:20.
  * Indices come out of a [128, 512] bits tile (rows 0:10 live, rows 10:128
    zeroed once) via a standard K=128 matmul against a zero-padded powers
    vector - one [128, 1] per-partition index tile per batch tile, no
    transposes, no small-K matmuls.

Sharding: data-parallel over batch across 8 cores (2048 rows each);
weights + memory table replicated. All operands are pre-transposed /
pre-tiled on the host so the device does zero transposes:
  - fp8 activations as [kp, bt, c4, ki2, bp] tiles (K = c*256 + ki*128 + kp)
  - fp8 weights as [kp, c4, ki2, n]
  - bf16 logit activations as [kp, g, kc, tb, bp] (one 8KB/partition DMA
    per 4-tile group), logit weights as [kp, kc8, 20]
Output is written bf16 (halves output DMA) and cast to fp32 on the host.
"""

import numpy as np
import ml_dtypes
from contextlib import ExitStack

import concourse.bass as bass
import concourse.mybir as mybir
import concourse.tile as tile
from concourse import bacc
from concourse import bass_utils

P = 128            # partitions
NCORES = 8
B = 16384          # full batch
BC = B // NCORES   # batch rows per core (2048)
BT = BC // P       # b-tiles per core (16)
C4 = 4             # fp8 DoubleRow contraction chunks (1024 / 256)
KC = 8             # bf16 contraction chunks (1024 / 128)
H = 1024
NB = 10            # address bits
MEM = 1024         # memory rows
LN_EPS = 1e-5
GROUP = 4          # b-tiles per logits group
NG = BT // GROUP   # logits groups (4)
WS = 16.0          # host-side weight scale for fp8

F32 = mybir.dt.float32
BF16 = mybir.dt.bfloat16
F8 = mybir.dt.float8e4
I32 = mybir.dt.int32
BF16_NP = ml_dtypes.bfloat16
F8_NP = ml_dtypes.float8_e4m3
DR = mybir.MatmulPerfMode.DoubleRow

_CACHE = {}


def _bcast_ap(handle, n):
    """[n] DRAM tensor -> [P, n] AP broadcast across partitions (step 0)."""
    h = handle.ap()
    return bass.AP(tensor=h.tensor, offset=h.offset, ap=[[0, P], *list(h.ap)])


def build_nc(zero_affine=True, warmup=True):
    nc = bacc.Bacc("TRN2", debug=False, enable_asserts=False)

    x8 = nc.dram_tensor("x8", [P, BT, C4, 2, P], F8, kind="ExternalInput")
    h8 = nc.dram_tensor("h8", [P, BT, C4, 2, P], F8, kind="ExternalInput")
    hh = nc.dram_tensor("hh", [P, NG, KC, GROUP, P], BF16, kind="ExternalInput")
    hl = nc.dram_tensor("hl", [P, NG, KC, GROUP, P], BF16, kind="ExternalInput")
    W8 = nc.dram_tensor("W8", [P, C4, 2, H], F8, kind="ExternalInput")
    U8 = nc.dram_tensor("U8", [P, C4, 2, H], F8, kind="ExternalInput")
    Q8 = nc.dram_tensor("Q8", [P, C4, 2, H], F8, kind="ExternalInput")
    M8 = nc.dram_tensor("M8", [P, C4, 2, MEM], F8, kind="ExternalInput")
    # [M_hi | zeros | M_lo] stacked columns: M_lo lands at partition 32 of
    # the logits PSUM tile (engine reads must start at partition 0/32/64/96).
    MSTK = 32 + NB
    Mstk = nc.dram_tensor("Mstk", [P, KC, MSTK], BF16, kind="ExternalInput")
    cb = nc.dram_tensor("cb", [H], F32, kind="ExternalInput")
    lng = nc.dram_tensor("lng", [H], F32, kind="ExternalInput")
    lnb = nc.dram_tensor("lnb", [H], F32, kind="ExternalInput")
    negmb = nc.dram_tensor("negmb", [NB], F32, kind="ExternalInput")
    powers = nc.dram_tensor("powers", [NB], F32, kind="ExternalInput")
    y = nc.dram_tensor("y", [BC, H], BF16, kind="ExternalOutput")
    R = nc.dram_tensor("Rtab", [MEM, H], BF16, kind="Internal")
    wsink_d = nc.dram_tensor("wsink", [P, 1], F32, kind="Internal")
    y_ap = y.ap()
    R_ap = R.ap()

    INV = 1.0 / WS

    with tile.TileContext(nc) as tc, ExitStack() as ctx:
        wpool = ctx.enter_context(tc.tile_pool(name="weights", bufs=1))
        work = ctx.enter_context(tc.tile_pool(name="work", bufs=4))
        hpool = ctx.enter_context(tc.tile_pool(name="hpool", bufs=2))
        epil = ctx.enter_context(tc.tile_pool(name="epil", bufs=GROUP))
        small = ctx.enter_context(tc.tile_pool(name="small", bufs=2 * GROUP + 2))
        psum = ctx.enter_context(tc.tile_pool(name="psum", bufs=2, space="PSUM"))

        wu_l = wpool.tile([P, P], BF16)
        wu_r = wpool.tile([P, 512], BF16)
        nc.vector.memset(wu_l[:], 0)
        nc.vector.memset(wu_r[:], 0)

        def pe_heat(n, tag):
            """n back-to-back dummy matmuls (one accumulation group) to keep
            the HAM clock gate at K=8/8 while the PE has no real work."""
            ps_w = psum.tile([P, 512], F32, tag="psT", space="PSUM", bufs=1,
                             name=tag)
            for i in range(n):
                nc.tensor.matmul(out=ps_w[:], lhsT=wu_l[:], rhs=wu_r[:],
                                 start=(i == 0), stop=(i == n - 1))
            wsink = wpool.tile([P, 1], F32, name=f"ws_{tag}")
            nc.vector.tensor_copy(out=wsink[:], in_=ps_w[:, 0:1])
            nc.sync.dma_start(out=wsink_d.ap()[:, :], in_=wsink[:])

        if warmup:
            # garbage SBUF values are fine; the sink result is discarded
            pe_heat(20, "wu0")

        # ---- resident constants; R operands chunked so matmuls start early
        m8_sb = wpool.tile([P, C4, 2, MEM], F8)
        q8_sb = wpool.tile([P, C4, 2, H], F8)
        w8_sb = wpool.tile([P, C4, 2, H], F8)
        u8_sb = wpool.tile([P, C4, 2, H], F8)
        mstk_sb = wpool.tile([P, KC, MSTK], BF16)
        nc.sync.dma_start(out=mstk_sb[:], in_=Mstk.ap()[:, :, :])
        for c in range(C4):
            nc.sync.dma_start(out=m8_sb[:, c, :, :], in_=M8.ap()[:, c, :, :])
            nc.sync.dma_start(out=q8_sb[:, c, :, :], in_=Q8.ap()[:, c, :, :])

        for c in range(C4):
            nc.sync.dma_start(out=w8_sb[:, c, :, :], in_=W8.ap()[:, c, :, :])
        for c in range(C4):
            nc.sync.dma_start(out=u8_sb[:, c, :, :], in_=U8.ap()[:, c, :, :])

        cbb = wpool.tile([P, H], F32)
        nc.gpsimd.dma_start(out=cbb[:], in_=_bcast_ap(cb, H))
        nmb_c = wpool.tile([NB, 1], F32)
        nc.sync.dma_start(out=nmb_c[:], in_=negmb.ap()[:, None])
        pw128 = wpool.tile([P, 1], F32)
        nc.vector.memset(pw128[:], 0)
        nc.sync.dma_start(out=pw128[0:NB, :], in_=powers.ap()[:, None])
        bits = wpool.tile([P, 512], F32)
        nc.vector.memset(bits[:], 0)
        if not zero_affine:
            gb = wpool.tile([P, H], F32)
            bb = wpool.tile([P, H], F32)
            nc.gpsimd.dma_start(out=gb[:], in_=_bcast_ap(lng, H))
            nc.gpsimd.dma_start(out=bb[:], in_=_bcast_ap(lnb, H))
            eps = wpool.tile([P, 1], F32)
            nc.vector.memset(eps[:], LN_EPS)

        # ---- phase 1: R = (memory @ Q_w.T)/WS + combined_bias -> DRAM bf16
        for mt in range(KC):
            psA = psum.tile([P, 512], F32, tag="psA", space="PSUM", bufs=3)
            psB = psum.tile([P, 512], F32, tag="psB", space="PSUM", bufs=3)
            for c in range(C4):
                lhs = m8_sb[:, c, :, mt * P:(mt + 1) * P]
                nc.tensor.matmul(out=psA[:], lhsT=lhs,
                                 rhs=q8_sb[:, c, :, 0:512],
                                 start=(c == 0), stop=(c == C4 - 1),
                                 perf_mode=DR)
                nc.tensor.matmul(out=psB[:], lhsT=lhs,
                                 rhs=q8_sb[:, c, :, 512:1024],
                                 start=(c == 0), stop=(c == C4 - 1),
                                 perf_mode=DR)
            r_sb = work.tile([P, H], BF16, tag="rtile")
            nc.vector.scalar_tensor_tensor(out=r_sb[:, 0:512], in0=psA[:],
                                           scalar=INV, in1=cbb[:, 0:512],
                                           op0=mybir.AluOpType.mult,
                                           op1=mybir.AluOpType.add)
            nc.vector.scalar_tensor_tensor(out=r_sb[:, 512:1024], in0=psB[:],
                                           scalar=INV, in1=cbb[:, 512:1024],
                                           op0=mybir.AluOpType.mult,
                                           op1=mybir.AluOpType.add)
            nc.gpsimd.dma_start(out=R_ap[mt * P:(mt + 1) * P, :], in_=r_sb[:])

        # ---- phase 2 ----
        def logits_group(g):
            """Exact fp32 logits for GROUP b-tiles -> [P, GROUP] index tile."""
            # per-kc chunk DMAs (128KB each): the first matmul starts as soon
            # as the first chunk lands instead of waiting out a 1MB burst
            hgh = hpool.tile([P, KC, GROUP, P], BF16, tag="hgh")
            hgl = hpool.tile([P, KC, GROUP, P], BF16, tag="hgl")
            for kc in range(KC):
                nc.sync.dma_start(out=hgh[:, kc, :, :],
                                  in_=hh.ap()[:, g, kc, :, :])
                nc.sync.dma_start(out=hgl[:, kc, :, :],
                                  in_=hl.ap()[:, g, kc, :, :])
            psLT = psum.tile([MSTK, 512], F32, tag="psL", space="PSUM",
                             bufs=1)
            for kc in range(KC):
                nc.tensor.matmul(out=psLT[:], lhsT=mstk_sb[:, kc, :],
                                 rhs=hgh[:, kc, :, :], start=(kc == 0),
                                 stop=False)
            for kc in range(KC):
                nc.tensor.matmul(out=psLT[:], lhsT=mstk_sb[:, kc, :],
                                 rhs=hgl[:, kc, :, :], start=False,
                                 stop=(kc == KC - 1))
            # DVE may read only one PSUM operand per op: stage the M_lo rows
            # through SBUF, then add against the M_hi rows still in PSUM.
            lo_sb = small.tile([NB, 512], F32, tag="lo_sb")
            nc.vector.tensor_copy(out=lo_sb[:], in_=psLT[32:32 + NB, :])
            lg = small.tile([NB, 512], F32, tag="lg")
            nc.vector.tensor_tensor(out=lg[:], in0=psLT[0:NB, :],
                                    in1=lo_sb[:],
                                    op=mybir.AluOpType.add)
            # bits rows 0:10 (rows 10:128 pre-zeroed; pw128 rows 10:128 = 0)
            nc.vector.tensor_scalar(out=bits[0:NB, :], in0=lg[:],
                                    scalar1=nmb_c[:], scalar2=None,
                                    op0=mybir.AluOpType.is_gt)
            # per-tile indices into 4 columns of one PSUM tile (no PE<->DVE
            # ping-pong), one i32 copy for the whole group
            pst = psum.tile([P, GROUP], F32, tag="psT", space="PSUM", bufs=1)
            for tb in range(GROUP):
                nc.tensor.matmul(out=pst[:, tb:tb + 1],
                                 lhsT=bits[:, tb * P:(tb + 1) * P],
                                 rhs=pw128[:], start=True, stop=True)
            idx = small.tile([P, GROUP], I32, tag="idx")
            nc.vector.tensor_copy(out=idx[:], in_=pst[:])
            return idx

        def stage_a(bt, idx_ap):
            xb = work.tile([P, C4, 2, P], F8, tag="xb")
            hb = work.tile([P, C4, 2, P], F8, tag="hb")
            nc.sync.dma_start(out=xb[:], in_=x8.ap()[:, bt, :, :, :])
            nc.sync.dma_start(out=hb[:], in_=h8.ap()[:, bt, :, :, :])

            rg = work.tile([P, H], BF16, tag="rg")
            nc.gpsimd.indirect_dma_start(
                out=rg[:], out_offset=None, in_=R_ap[:, :],
                in_offset=bass.IndirectOffsetOnAxis(ap=idx_ap, axis=0))

            ps0 = psum.tile([P, 512], F32, tag="psA", space="PSUM", bufs=3)
            ps1 = psum.tile([P, 512], F32, tag="psB", space="PSUM", bufs=3)
            for c in range(C4):
                nc.tensor.matmul(out=ps0[:], lhsT=xb[:, c, :, :],
                                 rhs=w8_sb[:, c, :, 0:512],
                                 start=(c == 0), stop=False, perf_mode=DR)
                nc.tensor.matmul(out=ps1[:], lhsT=xb[:, c, :, :],
                                 rhs=w8_sb[:, c, :, 512:1024],
                                 start=(c == 0), stop=False, perf_mode=DR)
            for c in range(C4):
                nc.tensor.matmul(out=ps0[:], lhsT=hb[:, c, :, :],
                                 rhs=u8_sb[:, c, :, 0:512],
                                 start=False, stop=(c == C4 - 1), perf_mode=DR)
                nc.tensor.matmul(out=ps1[:], lhsT=hb[:, c, :, :],
                                 rhs=u8_sb[:, c, :, 512:1024],
                                 start=False, stop=(c == C4 - 1), perf_mode=DR)

            pre = epil.tile([P, H], F32, tag="pre")
            nc.vector.scalar_tensor_tensor(out=pre[:, 0:512], in0=ps0[:],
                                           scalar=INV, in1=rg[:, 0:512],
                                           op0=mybir.AluOpType.mult,
                                           op1=mybir.AluOpType.add)
            nc.vector.scalar_tensor_tensor(out=pre[:, 512:1024], in0=ps1[:],
                                           scalar=INV, in1=rg[:, 512:1024],
                                           op0=mybir.AluOpType.mult,
                                           op1=mybir.AluOpType.add)

            stats = small.tile([P, 2, 6], F32, tag="stats")
            mv = small.tile([P, 2], F32, tag="mv")
            nc.vector.bn_stats(out=stats[:, 0, :], in_=pre[:, 0:512])
            nc.vector.bn_stats(out=stats[:, 1, :], in_=pre[:, 512:1024])
            nc.vector.bn_aggr(out=mv[:], in_=stats[:])

            if zero_affine:
                # rstd via bit-trick + 2 Newton steps (keeps Scalar engine's
                # activation table pinned to Sigmoid); then one fused
                # activation: sigmoid(pre * rstd - mu * rstd).
                v = small.tile([P, 1], F32, tag="v")
                ri = small.tile([P, 1], I32, tag="ri")
                t = small.tile([P, 1], F32, tag="t")
                nmr = small.tile([P, 1], F32, tag="nmr")
                ry = ri[:].bitcast(F32)
                nc.vector.tensor_scalar_add(out=v[:], in0=mv[:, 1:2],
                                            scalar1=LN_EPS)
                nc.vector.tensor_scalar(out=ri[:], in0=v[:].bitcast(I32),
                                        scalar1=1, scalar2=None,
                                        op0=mybir.AluOpType.arith_shift_right)
                nc.vector.tensor_scalar(out=ri[:], in0=ri[:], scalar1=0,
                                        scalar2=None,
                                        op0=mybir.AluOpType.bitwise_not)
                nc.vector.tensor_scalar(out=ri[:], in0=ri[:],
                                        scalar1=0x5F3759E0, scalar2=None,
                                        op0=mybir.AluOpType.add)
                for _ in range(2):
                    nc.vector.tensor_tensor(out=t[:], in0=ry, in1=ry,
                                            op=mybir.AluOpType.mult)
                    nc.vector.tensor_tensor(out=t[:], in0=t[:], in1=v[:],
                                            op=mybir.AluOpType.mult)
                    nc.vector.tensor_scalar(out=t[:], in0=t[:], scalar1=-0.5,
                                            scalar2=1.5,
                                            op0=mybir.AluOpType.mult,
                                            op1=mybir.AluOpType.add)
                    nc.vector.tensor_tensor(out=ry, in0=ry, in1=t[:],
                                            op=mybir.AluOpType.mult)
                nc.vector.scalar_tensor_tensor(out=nmr[:], in0=mv[:, 0:1],
                                               scalar=-1.0, in1=ry,
                                               op0=mybir.AluOpType.mult,
                                               op1=mybir.AluOpType.mult)
                ob = work.tile([P, H], BF16, tag="ob")
                nc.scalar.activation(out=ob[:], in_=pre[:],
                                     func=mybir.ActivationFunctionType.Sigmoid,
                                     bias=nmr[:], scale=ri[:].bitcast(F32))
                nc.sync.dma_start(out=y_ap[bt * P:(bt + 1) * P, :], in_=ob[:])
                return None

            sd = small.tile([P, 1], F32, tag="sd")
            rstd = small.tile([P, 1], F32, tag="rstd")
            nc.scalar.activation(out=sd[:], in_=mv[:, 1:2],
                                 func=mybir.ActivationFunctionType.Sqrt,
                                 bias=eps[:], scale=1.0)
            nc.vector.reciprocal(out=rstd[:], in_=sd[:])
            return pre, mv, rstd

        def stage_b(bt, pre, mv, rstd):
            nc.vector.scalar_tensor_tensor(out=pre[:], in0=pre[:],
                                           scalar=mv[:, 0:1], in1=gb[:],
                                           op0=mybir.AluOpType.subtract,
                                           op1=mybir.AluOpType.mult)
            nc.vector.scalar_tensor_tensor(out=pre[:], in0=pre[:],
                                           scalar=rstd[:], in1=bb[:],
                                           op0=mybir.AluOpType.mult,
                                           op1=mybir.AluOpType.add)
            ob = work.tile([P, H], BF16, tag="ob")
            nc.scalar.activation(out=ob[:], in_=pre[:],
                                 func=mybir.ActivationFunctionType.Sigmoid)
            nc.sync.dma_start(out=y_ap[bt * P:(bt + 1) * P, :], in_=ob[:])

        for g in range(NG):
            idx = logits_group(g)
            staged = []
            for tb in range(GROUP):
                r = stage_a(g * GROUP + tb, idx[:, tb:tb + 1])
                if r is not None:
                    staged.append((g * GROUP + tb, *r))
            for bt, pre, mv, rstd in staged:
                stage_b(bt, pre, mv, rstd)

    nc.compile()
    return nc


def _get_nc(zero_affine=True):
    key = ("nc", zero_affine)
    if key not in _CACHE:
        _CACHE[key] = build_nc(zero_affine)
    return _CACHE[key]


def _tile_act8(a):
    """[BC, 1024] f32 -> [kp, bt, c4, ki2, bp] fp8, k = c*256 + ki*128 + kp."""
    t = a.reshape(BT, P, C4, 2, P).transpose(4, 0, 2, 3, 1)
    return np.ascontiguousarray(t).astype(F8_NP)


def _tile_w8(w):
    """[n, 1024] f32 (contraction on axis 1) -> [kp, c4, ki2, n] fp8."""
    t = w.T.reshape(C4, 2, P, -1).transpose(2, 0, 1, 3)
    return np.ascontiguousarray(t).astype(F8_NP)


def _tile_hpair(a):
    """[BC, 1024] bf16 -> [kp, g, kc, tb, bp], k = kc*128 + kp."""
    t = a.reshape(NG, GROUP, P, KC, P).transpose(4, 0, 3, 1, 2)
    return np.ascontiguousarray(t)


def prepare_in_maps(inputs):
    x = np.asarray(inputs["x"], np.float32)
    h = np.asarray(inputs["h_prev"], np.float32)
    memory = np.asarray(inputs["memory"], np.float32)
    W_w = np.asarray(inputs["W_w"], np.float32)
    U_w = np.asarray(inputs["U_w"], np.float32)
    Q_w = np.asarray(inputs["Q_w"], np.float32)
    M_w = np.asarray(inputs["M_w"], np.float32)
    W_b = np.asarray(inputs["W_b"], np.float32)
    U_b = np.asarray(inputs["U_b"], np.float32)
    Q_b = np.asarray(inputs["Q_b"], np.float32)
    M_b = np.asarray(inputs["M_b"], np.float32)
    ln_g = np.asarray(inputs["ln_g"], np.float32)
    ln_b = np.asarray(inputs["ln_b"], np.float32)

    M_hi = M_w.astype(BF16_NP).astype(np.float32)
    M_lo = (M_w - M_hi).astype(BF16_NP).astype(np.float32)
    # [kp, kc, 42] = [M_hi | zeros(22) | M_lo] columns, k = kc*128 + kp
    mstk = np.concatenate(
        [M_hi.T, np.zeros((H, 22), np.float32), M_lo.T], axis=1)  # [1024, 42]
    mstk = mstk.reshape(KC, P, 42).transpose(1, 0, 2)

    shared = {
        "W8": _tile_w8(W_w * WS),
        "U8": _tile_w8(U_w * WS),
        "Q8": _tile_w8(Q_w * WS),
        # contraction for R = memory @ Q_w.T is over memory's axis 1 (HIDDEN);
        # rows (axis 0) are the "out" dim -> same transform as W.
        "M8": _tile_w8(memory),
        "Mstk": np.ascontiguousarray(mstk).astype(BF16_NP),
        "cb": np.ascontiguousarray(W_b + U_b + Q_b),
        "lng": np.ascontiguousarray(ln_g),
        "lnb": np.ascontiguousarray(ln_b),
        "negmb": np.ascontiguousarray(-M_b),
        "powers": (2.0 ** np.arange(NB - 1, -1, -1)).astype(np.float32),
    }
    in_maps = []
    for i in range(NCORES):
        sl = slice(i * BC, (i + 1) * BC)
        hs = h[sl]
        h_hi = hs.astype(BF16_NP)
        h_lo = (hs - h_hi.astype(np.float32)).astype(BF16_NP)
        m = dict(shared)
        m["x8"] = _tile_act8(x[sl])
        m["h8"] = _tile_act8(hs)
        m["hh"] = _tile_hpair(h_hi)
        m["hl"] = _tile_hpair(h_lo)
        in_maps.append(m)
    return in_maps


def run(inputs, trace=False, trace_cores=None):
    zero_affine = bool(
        np.all(np.asarray(inputs["ln_g"], np.float32) == 1.0)
        and np.all(np.asarray(inputs["ln_b"], np.float32) == 0.0))
    nc = _get_nc(zero_affine)
    in_maps = prepare_in_maps(inputs)
    res = bass_utils.run_bass_kernel_spmd(
        nc, in_maps, core_ids=list(range(NCORES)), trace=trace,
        trace_cores=trace_cores)
    out = np.concatenate(
        [np.asarray(r["y"]).astype(np.float32) for r in res.results], axis=0)
    return out, res


def kernel(**inputs):
    out, _ = run(inputs)
    return out.astype(np.float32)


def enable_profiling():
    """Inject the missing antenv.axon_hooks shim so trace=True works, and
    neutralize the S3 artifact upload (zero-egress container)."""
    import sys
    import types
    try:
        import antenv.axon_hooks  # noqa: F401
    except ImportError:
        mod = types.ModuleType("antenv.axon_hooks")
        _hook = [None]
        mod.set_axon_ntff_profile_hook = lambda h: _hook.__setitem__(0, h)
        mod.get_axon_ntff_profile_hook = lambda: _hook[0]
        sys.modules["antenv.axon_hooks"] = mod
        from trn_agent_boot.trn_boot import _ntff_profile_via_ctypes
        mod.set_axon_ntff_profile_hook(
            _ntff_profile_via_ctypes("/opt/axon/libaxon_pjrt.so"))
    bass_utils.upload_artifacts = lambda d: "local://" + str(d)


# revision 11
# speedup vs baseline: 1.2610x; 1.1104x over previous
"""Trainium2 Bass kernel for BinaryMemoryRNN (scatter_memory).

Math (per batch row b):
    logits = h_prev @ M_w.T + M_b                 [B, 10]
    bits   = (sigmoid(logits) > 0.5) = (logits > -M_b)
    index  = sum(bits * 2^(9-i))                  [B] in [0, 1023]
    h_mem  = memory[index]
    pre    = x @ W_w.T + W_b + h_prev @ U_w.T + U_b + h_mem @ Q_w.T + Q_b
    out    = sigmoid(LayerNorm(pre) * ln_g + ln_b)

Key transforms:
  * h_mem @ Q_w.T == (memory @ Q_w.T)[index], so R = memory @ Q_w.T + bias
    is precomputed once per core ([1024, 1024]) and the gather+matmul becomes
    a row gather of R feeding an add.
  * The three big matmuls (x@W.T, h@U.T, memory@Q.T) run in fp8 e4m3 with
    DoubleRow perf mode (2x PE throughput vs bf16). Weights are scaled by 16
    on the host so their ~0.02-magnitude entries stay in e4m3's normal range;
    the 1/16 descale is fused into the PSUM-evacuating add. Simulated end to
    end this lands at rel err ~1.2e-2 vs the fp32 reference (gate: 2e-2).
  * The address logits must be near-fp32 exact (a flipped bit selects a
    completely different memory row, ~2.5e-3 rel err per flip). They are
    computed as (h_hi + h_lo) @ (M_hi + M_lo) with all four factors bf16 and
    fp32 PSUM accumulation: bf16 products are exact in fp32, so the only loss
    is the h_lo rounding (~2^-17 relative) - simulation shows 0 flipped
    indices. Grouped transposed form: lhsT = [M_hi | M_lo] stacked [128, 20],
    rhs = 512 batch columns, accumulating h_hi then h_lo streams into a
    [20, 512] PSUM tile; logits = rows 0:10 + rows 10:20.
  * Indices come out of a [128, 512] bits tile (rows 0:10 live, rows 10:128
    zeroed once) via a standard K=128 matmul against a zero-padded powers
    vector - one [128, 1] per-partition index tile per batch tile, no
    transposes, no small-K matmuls.

Sharding: data-parallel over batch across 8 cores (2048 rows each);
weights + memory table replicated. All operands are pre-transposed /
pre-tiled on the host so the device does zero transposes:
  - fp8 activations as [kp, bt, c4, ki2, bp] tiles (K = c*256 + ki*128 + kp)
  - fp8 weights as [kp, c4, ki2, n]
  - bf16 logit activations as [kp, g, kc, tb, bp] (one 8KB/partition DMA
    per 4-tile group), logit weights as [kp, kc8, 20]
Output is written bf16 (halves output DMA) and cast to fp32 on the host.
"""

import numpy as np
import ml_dtypes
from contextlib import ExitStack

import concourse.bass as bass
import concourse.mybir as mybir
import concourse.tile as tile
from concourse import bacc
from concourse import bass_utils

P = 128            # partitions
NCORES = 8
B = 16384          # full batch
BC = B // NCORES   # batch rows per core (2048)
BT = BC // P       # b-tiles per core (16)
C4 = 4             # fp8 DoubleRow contraction chunks (1024 / 256)
KC = 8             # bf16 contraction chunks (1024 / 128)
H = 1024
NB = 10            # address bits
MEM = 1024         # memory rows
LN_EPS = 1e-5
GROUP = 4          # b-tiles per logits group
NG = BT // GROUP   # logits groups (4)
WS = 16.0          # host-side weight scale for fp8

F32 = mybir.dt.float32
BF16 = mybir.dt.bfloat16
F8 = mybir.dt.float8e4
I32 = mybir.dt.int32
BF16_NP = ml_dtypes.bfloat16
F8_NP = ml_dtypes.float8_e4m3
DR = mybir.MatmulPerfMode.DoubleRow

_CACHE = {}


def _bcast_ap(handle, n):
    """[n] DRAM tensor -> [P, n] AP broadcast across partitions (step 0)."""
    h = handle.ap()
    return bass.AP(tensor=h.tensor, offset=h.offset, ap=[[0, P], *list(h.ap)])


def build_nc(zero_affine=True, warmup=True):
    nc = bacc.Bacc("TRN2", debug=False, enable_asserts=False)

    x8 = nc.dram_tensor("x8", [P, BT, C4, 2, P], F8, kind="ExternalInput")
    h8 = nc.dram_tensor("h8", [P, BT, C4, 2, P], F8, kind="ExternalInput")
    hh = nc.dram_tensor("hh", [P, NG, KC, GROUP, P], BF16, kind="ExternalInput")
    hl = nc.dram_tensor("hl", [P, NG, KC, GROUP, P], BF16, kind="ExternalInput")
    W8 = nc.dram_tensor("W8", [P, C4, 2, H], F8, kind="ExternalInput")
    U8 = nc.dram_tensor("U8", [P, C4, 2, H], F8, kind="ExternalInput")
    Q8 = nc.dram_tensor("Q8", [P, C4, 2, H], F8, kind="ExternalInput")
    M8 = nc.dram_tensor("M8", [P, C4, 2, MEM], F8, kind="ExternalInput")
    # [M_hi | zeros | M_lo] stacked columns: M_lo lands at partition 32 of
    # the logits PSUM tile (engine reads must start at partition 0/32/64/96).
    MSTK = 32 + NB
    Mstk = nc.dram_tensor("Mstk", [P, KC, MSTK], BF16, kind="ExternalInput")
    cb = nc.dram_tensor("cb", [H], F32, kind="ExternalInput")
    lng = nc.dram_tensor("lng", [H], F32, kind="ExternalInput")
    lnb = nc.dram_tensor("lnb", [H], F32, kind="ExternalInput")
    negmb = nc.dram_tensor("negmb", [NB], F32, kind="ExternalInput")
    powers = nc.dram_tensor("powers", [NB], F32, kind="ExternalInput")
    y = nc.dram_tensor("y", [BC, H], BF16, kind="ExternalOutput")
    R = nc.dram_tensor("Rtab", [MEM, H], BF16, kind="Internal")
    wsink_d = nc.dram_tensor("wsink", [P, 1], F32, kind="Internal")
    y_ap = y.ap()
    R_ap = R.ap()

    INV = 1.0 / WS

    with tile.TileContext(nc) as tc, ExitStack() as ctx:
        wpool = ctx.enter_context(tc.tile_pool(name="weights", bufs=1))
        work = ctx.enter_context(tc.tile_pool(name="work", bufs=4))
        hpool = ctx.enter_context(tc.tile_pool(name="hpool", bufs=2))
        epil = ctx.enter_context(tc.tile_pool(name="epil", bufs=GROUP))
        small = ctx.enter_context(tc.tile_pool(name="small", bufs=2 * GROUP + 2))
        psum = ctx.enter_context(tc.tile_pool(name="psum", bufs=2, space="PSUM"))

        wu_l = wpool.tile([P, P], BF16)
        wu_r = wpool.tile([P, 512], BF16)
        nc.vector.memset(wu_l[:], 0)
        nc.vector.memset(wu_r[:], 0)

        ps_w = psum.tile([P, 512], F32, tag="psT", space="PSUM", bufs=1)

        def pe_heat(n, tag, sink):
            """n back-to-back dummy matmuls (one accumulation group) into the
            shared ps_w tile: keeps the PE p-state at full clock while it has
            no real work (any idle gap costs ~3.4us of half-clock on resume).
            """
            for i in range(n):
                nc.tensor.matmul(out=ps_w[:], lhsT=wu_l[:], rhs=wu_r[:],
                                 start=(i == 0), stop=(i == n - 1))
            if sink:
                wsink = wpool.tile([P, 1], F32, name=f"ws_{tag}")
                nc.vector.tensor_copy(out=wsink[:], in_=ps_w[:, 0:1])
                nc.sync.dma_start(out=wsink_d.ap()[:, :], in_=wsink[:])

        if warmup:
            # ramp low->mid->full p-state on dummy work while DMAs prefill
            pe_heat(25, "wu0", sink=True)

        # ---- resident constants; R operands chunked so matmuls start early
        m8_sb = wpool.tile([P, C4, 2, MEM], F8)
        q8_sb = wpool.tile([P, C4, 2, H], F8)
        w8_sb = wpool.tile([P, C4, 2, H], F8)
        u8_sb = wpool.tile([P, C4, 2, H], F8)
        mstk_sb = wpool.tile([P, KC, MSTK], BF16)
        nc.sync.dma_start(out=mstk_sb[:], in_=Mstk.ap()[:, :, :])
        for c in range(C4):
            nc.sync.dma_start(out=m8_sb[:, c, :, :], in_=M8.ap()[:, c, :, :])
            nc.sync.dma_start(out=q8_sb[:, c, :, :], in_=Q8.ap()[:, c, :, :])

        for c in range(C4):
            nc.sync.dma_start(out=w8_sb[:, c, :, :], in_=W8.ap()[:, c, :, :])
        for c in range(C4):
            nc.sync.dma_start(out=u8_sb[:, c, :, :], in_=U8.ap()[:, c, :, :])

        cbb = wpool.tile([P, H], F32)
        nc.gpsimd.dma_start(out=cbb[:], in_=_bcast_ap(cb, H))
        nmb_c = wpool.tile([NB, 1], F32)
        nc.sync.dma_start(out=nmb_c[:], in_=negmb.ap()[:, None])
        pw128 = wpool.tile([P, 1], F32)
        nc.vector.memset(pw128[:], 0)
        nc.sync.dma_start(out=pw128[0:NB, :], in_=powers.ap()[:, None])
        bits = wpool.tile([P, 512], F32)
        nc.vector.memset(bits[:], 0)
        if not zero_affine:
            gb = wpool.tile([P, H], F32)
            bb = wpool.tile([P, H], F32)
            nc.gpsimd.dma_start(out=gb[:], in_=_bcast_ap(lng, H))
            nc.gpsimd.dma_start(out=bb[:], in_=_bcast_ap(lnb, H))
            eps = wpool.tile([P, 1], F32)
            nc.vector.memset(eps[:], LN_EPS)

        # ---- phase 1: R = (memory @ Q_w.T)/WS + combined_bias -> DRAM bf16
        for mt in range(KC):
            psA = psum.tile([P, 512], F32, tag="psA", space="PSUM", bufs=3)
            psB = psum.tile([P, 512], F32, tag="psB", space="PSUM", bufs=3)
            for c in range(C4):
                lhs = m8_sb[:, c, :, mt * P:(mt + 1) * P]
                nc.tensor.matmul(out=psA[:], lhsT=lhs,
                                 rhs=q8_sb[:, c, :, 0:512],
                                 start=(c == 0), stop=(c == C4 - 1),
                                 perf_mode=DR)
                nc.tensor.matmul(out=psB[:], lhsT=lhs,
                                 rhs=q8_sb[:, c, :, 512:1024],
                                 start=(c == 0), stop=(c == C4 - 1),
                                 perf_mode=DR)
            r_sb = work.tile([P, H], BF16, tag="rtile")
            nc.vector.scalar_tensor_tensor(out=r_sb[:, 0:512], in0=psA[:],
                                           scalar=INV, in1=cbb[:, 0:512],
                                           op0=mybir.AluOpType.mult,
                                           op1=mybir.AluOpType.add)
            nc.vector.scalar_tensor_tensor(out=r_sb[:, 512:1024], in0=psB[:],
                                           scalar=INV, in1=cbb[:, 512:1024],
                                           op0=mybir.AluOpType.mult,
                                           op1=mybir.AluOpType.add)
            nc.gpsimd.dma_start(out=R_ap[mt * P:(mt + 1) * P, :], in_=r_sb[:])

        # ---- phase 2 ----
        def logits_group(g):
            """Exact fp32 logits for GROUP b-tiles -> [P, GROUP] index tile."""
            hgh = hpool.tile([P, KC, GROUP, P], BF16, tag="hgh")
            hgl = hpool.tile([P, KC, GROUP, P], BF16, tag="hgl")
            nc.sync.dma_start(out=hgh[:], in_=hh.ap()[:, g, :, :, :])
            nc.sync.dma_start(out=hgl[:], in_=hl.ap()[:, g, :, :, :])
            psLT = psum.tile([MSTK, 512], F32, tag="psL", space="PSUM",
                             bufs=1)
            for kc in range(KC):
                nc.tensor.matmul(out=psLT[:], lhsT=mstk_sb[:, kc, :],
                                 rhs=hgh[:, kc, :, :], start=(kc == 0),
                                 stop=False)
            for kc in range(KC):
                nc.tensor.matmul(out=psLT[:], lhsT=mstk_sb[:, kc, :],
                                 rhs=hgl[:, kc, :, :], start=False,
                                 stop=(kc == KC - 1))
            # DVE may read only one PSUM operand per op: stage the M_lo rows
            # through SBUF, then add against the M_hi rows still in PSUM.
            lo_sb = small.tile([NB, 512], F32, tag="lo_sb")
            nc.vector.tensor_copy(out=lo_sb[:], in_=psLT[32:32 + NB, :])
            lg = small.tile([NB, 512], F32, tag="lg")
            nc.vector.tensor_tensor(out=lg[:], in0=psLT[0:NB, :],
                                    in1=lo_sb[:],
                                    op=mybir.AluOpType.add)
            # bits rows 0:10 (rows 10:128 pre-zeroed; pw128 rows 10:128 = 0)
            nc.vector.tensor_scalar(out=bits[0:NB, :], in0=lg[:],
                                    scalar1=nmb_c[:], scalar2=None,
                                    op0=mybir.AluOpType.is_gt)
            # per-tile indices into 4 columns of one PSUM tile (no PE<->DVE
            # ping-pong), one i32 copy for the whole group
            pst = psum.tile([P, GROUP], F32, tag="psT", space="PSUM", bufs=1)
            for tb in range(GROUP):
                nc.tensor.matmul(out=pst[:, tb:tb + 1],
                                 lhsT=bits[:, tb * P:(tb + 1) * P],
                                 rhs=pw128[:], start=True, stop=True)
            idx = small.tile([P, GROUP], I32, tag="idx")
            nc.vector.tensor_copy(out=idx[:], in_=pst[:])
            return idx

        def stage_a(bt, idx_ap):
            xb = work.tile([P, C4, 2, P], F8, tag="xb")
            hb = work.tile([P, C4, 2, P], F8, tag="hb")
            nc.sync.dma_start(out=xb[:], in_=x8.ap()[:, bt, :, :, :])
            nc.sync.dma_start(out=hb[:], in_=h8.ap()[:, bt, :, :, :])

            rg = work.tile([P, H], BF16, tag="rg")
            nc.gpsimd.indirect_dma_start(
                out=rg[:], out_offset=None, in_=R_ap[:, :],
                in_offset=bass.IndirectOffsetOnAxis(ap=idx_ap, axis=0))

            ps0 = psum.tile([P, 512], F32, tag="psA", space="PSUM", bufs=3)
            ps1 = psum.tile([P, 512], F32, tag="psB", space="PSUM", bufs=3)
            for c in range(C4):
                nc.tensor.matmul(out=ps0[:], lhsT=xb[:, c, :, :],
                                 rhs=w8_sb[:, c, :, 0:512],
                                 start=(c == 0), stop=False, perf_mode=DR)
                nc.tensor.matmul(out=ps1[:], lhsT=xb[:, c, :, :],
                                 rhs=w8_sb[:, c, :, 512:1024],
                                 start=(c == 0), stop=False, perf_mode=DR)
            for c in range(C4):
                nc.tensor.matmul(out=ps0[:], lhsT=hb[:, c, :, :],
                                 rhs=u8_sb[:, c, :, 0:512],
                                 start=False, stop=(c == C4 - 1), perf_mode=DR)
                nc.tensor.matmul(out=ps1[:], lhsT=hb[:, c, :, :],
                                 rhs=u8_sb[:, c, :, 512:1024],
                                 start=False, stop=(c == C4 - 1), perf_mode=DR)

            pre = epil.tile([P, H], F32, tag="pre")
            nc.vector.scalar_tensor_tensor(out=pre[:, 0:512], in0=ps0[:],
                                           scalar=INV, in1=rg[:, 0:512],
                                           op0=mybir.AluOpType.mult,
                                           op1=mybir.AluOpType.add)
            nc.vector.scalar_tensor_tensor(out=pre[:, 512:1024], in0=ps1[:],
                                           scalar=INV, in1=rg[:, 512:1024],
                                           op0=mybir.AluOpType.mult,
                                           op1=mybir.AluOpType.add)

            stats = small.tile([P, 2, 6], F32, tag="stats")
            mv = small.tile([P, 2], F32, tag="mv")
            nc.vector.bn_stats(out=stats[:, 0, :], in_=pre[:, 0:512])
            nc.vector.bn_stats(out=stats[:, 1, :], in_=pre[:, 512:1024])
            nc.vector.bn_aggr(out=mv[:], in_=stats[:])

            if zero_affine:
                # rstd via bit-trick + 2 Newton steps (keeps Scalar engine's
                # activation table pinned to Sigmoid); then one fused
                # activation: sigmoid(pre * rstd - mu * rstd).
                v = small.tile([P, 1], F32, tag="v")
                ri = small.tile([P, 1], I32, tag="ri")
                t = small.tile([P, 1], F32, tag="t")
                nmr = small.tile([P, 1], F32, tag="nmr")
                ry = ri[:].bitcast(F32)
                nc.vector.tensor_scalar_add(out=v[:], in0=mv[:, 1:2],
                                            scalar1=LN_EPS)
                nc.vector.tensor_scalar(out=ri[:], in0=v[:].bitcast(I32),
                                        scalar1=1, scalar2=None,
                                        op0=mybir.AluOpType.arith_shift_right)
                nc.vector.tensor_scalar(out=ri[:], in0=ri[:], scalar1=0,
                                        scalar2=None,
                                        op0=mybir.AluOpType.bitwise_not)
                nc.vector.tensor_scalar(out=ri[:], in0=ri[:],
                                        scalar1=0x5F3759E0, scalar2=None,
                                        op0=mybir.AluOpType.add)
                for _ in range(2):
                    nc.vector.tensor_tensor(out=t[:], in0=ry, in1=ry,
                                            op=mybir.AluOpType.mult)
                    nc.vector.tensor_tensor(out=t[:], in0=t[:], in1=v[:],
                                            op=mybir.AluOpType.mult)
                    nc.vector.tensor_scalar(out=t[:], in0=t[:], scalar1=-0.5,
                                            scalar2=1.5,
                                            op0=mybir.AluOpType.mult,
                                            op1=mybir.AluOpType.add)
                    nc.vector.tensor_tensor(out=ry, in0=ry, in1=t[:],
                                            op=mybir.AluOpType.mult)
                nc.vector.scalar_tensor_tensor(out=nmr[:], in0=mv[:, 0:1],
                                               scalar=-1.0, in1=ry,
                                               op0=mybir.AluOpType.mult,
                                               op1=mybir.AluOpType.mult)
                ob = work.tile([P, H], BF16, tag="ob")
                nc.scalar.activation(out=ob[:], in_=pre[:],
                                     func=mybir.ActivationFunctionType.Sigmoid,
                                     bias=nmr[:], scale=ri[:].bitcast(F32))
                nc.sync.dma_start(out=y_ap[bt * P:(bt + 1) * P, :], in_=ob[:])
                return None

            sd = small.tile([P, 1], F32, tag="sd")
            rstd = small.tile([P, 1], F32, tag="rstd")
            nc.scalar.activation(out=sd[:], in_=mv[:, 1:2],
                                 func=mybir.ActivationFunctionType.Sqrt,
                                 bias=eps[:], scale=1.0)
            nc.vector.reciprocal(out=rstd[:], in_=sd[:])
            return pre, mv, rstd

        def stage_b(bt, pre, mv, rstd):
            nc.vector.scalar_tensor_tensor(out=pre[:], in0=pre[:],
                                           scalar=mv[:, 0:1], in1=gb[:],
                                           op0=mybir.AluOpType.subtract,
                                           op1=mybir.AluOpType.mult)
            nc.vector.scalar_tensor_tensor(out=pre[:], in0=pre[:],
                                           scalar=rstd[:], in1=bb[:],
                                           op0=mybir.AluOpType.mult,
                                           op1=mybir.AluOpType.add)
            ob = work.tile([P, H], BF16, tag="ob")
            nc.scalar.activation(out=ob[:], in_=pre[:],
                                 func=mybir.ActivationFunctionType.Sigmoid)
            nc.sync.dma_start(out=y_ap[bt * P:(bt + 1) * P, :], in_=ob[:])

        for g in range(NG):
            idx = logits_group(g)
            staged = []
            for tb in range(GROUP):
                r = stage_a(g * GROUP + tb, idx[:, tb:tb + 1])
                if r is not None:
                    staged.append((g * GROUP + tb, *r))
            for bt, pre, mv, rstd in staged:
                stage_b(bt, pre, mv, rstd)
        if warmup:
            # hold full clock while the last epilogues drain
            pe_heat(24, "wu1", sink=False)

    nc.compile()
    return nc


def _get_nc(zero_affine=True):
    key = ("nc", zero_affine)
    if key not in _CACHE:
        _CACHE[key] = build_nc(zero_affine)
    return _CACHE[key]


def _tile_act8(a):
    """[BC, 1024] f32 -> [kp, bt, c4, ki2, bp] fp8, k = c*256 + ki*128 + kp."""
    t = a.reshape(BT, P, C4, 2, P).transpose(4, 0, 2, 3, 1)
    return np.ascontiguousarray(t).astype(F8_NP)


def _tile_w8(w):
    """[n, 1024] f32 (contraction on axis 1) -> [kp, c4, ki2, n] fp8."""
    t = w.T.reshape(C4, 2, P, -1).transpose(2, 0, 1, 3)
    return np.ascontiguousarray(t).astype(F8_NP)


def _tile_hpair(a):
    """[BC, 1024] bf16 -> [kp, g, kc, tb, bp], k = kc*128 + kp."""
    t = a.reshape(NG, GROUP, P, KC, P).transpose(4, 0, 3, 1, 2)
    return np.ascontiguousarray(t)


def prepare_in_maps(inputs):
    x = np.asarray(inputs["x"], np.float32)
    h = np.asarray(inputs["h_prev"], np.float32)
    memory = np.asarray(inputs["memory"], np.float32)
    W_w = np.asarray(inputs["W_w"], np.float32)
    U_w = np.asarray(inputs["U_w"], np.float32)
    Q_w = np.asarray(inputs["Q_w"], np.float32)
    M_w = np.asarray(inputs["M_w"], np.float32)
    W_b = np.asarray(inputs["W_b"], np.float32)
    U_b = np.asarray(inputs["U_b"], np.float32)
    Q_b = np.asarray(inputs["Q_b"], np.float32)
    M_b = np.asarray(inputs["M_b"], np.float32)
    ln_g = np.asarray(inputs["ln_g"], np.float32)
    ln_b = np.asarray(inputs["ln_b"], np.float32)

    M_hi = M_w.astype(BF16_NP).astype(np.float32)
    M_lo = (M_w - M_hi).astype(BF16_NP).astype(np.float32)
    # [kp, kc, 42] = [M_hi | zeros(22) | M_lo] columns, k = kc*128 + kp
    mstk = np.concatenate(
        [M_hi.T, np.zeros((H, 22), np.float32), M_lo.T], axis=1)  # [1024, 42]
    mstk = mstk.reshape(KC, P, 42).transpose(1, 0, 2)

    shared = {
        "W8": _tile_w8(W_w * WS),
        "U8": _tile_w8(U_w * WS),
        "Q8": _tile_w8(Q_w * WS),
        # contraction for R = memory @ Q_w.T is over memory's axis 1 (HIDDEN);
        # rows (axis 0) are the "out" dim -> same transform as W.
        "M8": _tile_w8(memory),
        "Mstk": np.ascontiguousarray(mstk).astype(BF16_NP),
        "cb": np.ascontiguousarray(W_b + U_b + Q_b),
        "lng": np.ascontiguousarray(ln_g),
        "lnb": np.ascontiguousarray(ln_b),
        "negmb": np.ascontiguousarray(-M_b),
        "powers": (2.0 ** np.arange(NB - 1, -1, -1)).astype(np.float32),
    }
    in_maps = []
    for i in range(NCORES):
        sl = slice(i * BC, (i + 1) * BC)
        hs = h[sl]
        h_hi = hs.astype(BF16_NP)
        h_lo = (hs - h_hi.astype(np.float32)).astype(BF16_NP)
        m = dict(shared)
        m["x8"] = _tile_act8(x[sl])
        m["h8"] = _tile_act8(hs)
        m["hh"] = _tile_hpair(h_hi)
        m["hl"] = _tile_hpair(h_lo)
        in_maps.append(m)
    return in_maps


def run(inputs, trace=False, trace_cores=None):
    zero_affine = bool(
        np.all(np.asarray(inputs["ln_g"], np.float32) == 1.0)
        and np.all(np.asarray(inputs["ln_b"], np.float32) == 0.0))
    nc = _get_nc(zero_affine)
    in_maps = prepare_in_maps(inputs)
    res = bass_utils.run_bass_kernel_spmd(
        nc, in_maps, core_ids=list(range(NCORES)), trace=trace,
        trace_cores=trace_cores)
    out = np.concatenate(
        [np.asarray(r["y"]).astype(np.float32) for r in res.results], axis=0)
    return out, res


def kernel(**inputs):
    out, _ = run(inputs)
    return out.astype(np.float32)


def enable_profiling():
    """Inject the missing antenv.axon_hooks shim so trace=True works, and
    neutralize the S3 artifact upload (zero-egress container)."""
    import sys
    import types
    try:
        import antenv.axon_hooks  # noqa: F401
    except ImportError:
        mod = types.ModuleType("antenv.axon_hooks")
        _hook = [None]
        mod.set_axon_ntff_profile_hook = lambda h: _hook.__setitem__(0, h)
        mod.get_axon_ntff_profile_hook = lambda: _hook[0]
        sys.modules["antenv.axon_hooks"] = mod
        from trn_agent_boot.trn_boot import _ntff_profile_via_ctypes
        mod.set_axon_ntff_profile_hook(
            _ntff_profile_via_ctypes("/opt/axon/libaxon_pjrt.so"))
    bass_utils.upload_artifacts = lambda d: "local://" + str(d)
